# revision 25
# baseline (speedup 1.0000x reference)
"""Trainium2 Bass kernel: causal GQA self-attention (B=2, T=2048, C=1024,
16 q-heads / 4 kv-heads, rotary + q/k RMS-norm), sharded over 8 NeuronCores
as (batch x kv-group). Self-contained: kernel(**inputs) -> np.ndarray.

Single interleaved pipeline: fused QKV projection, grouped rope/RMS (norms
computed pre-rotation; rotation preserves them), software-pipelined causal
attention with column-trimmed scores/exp, and cross-group task interleaving
to keep the tensor engine ramped. Launch path uses fast-dispatch compile.
"""
import sys
from contextlib import ExitStack

for p in ("/opt/trn_rl_repo", "/root/.axon_site/_ro/trn_rl_repo"):
    if p not in sys.path:
        sys.path.insert(0, p)

import numpy as np
import ml_dtypes

import concourse.bass as bass
import concourse.mybir as mybir
from concourse.tile import TileContext
from concourse.masks import make_identity

F32 = mybir.dt.float32
BF16 = mybir.dt.bfloat16
NPBF16 = ml_dtypes.bfloat16

T, C, HQ, D = 2048, 1024, 4, 64
DQ = HQ * D          # 256 q dims per core
DKV = DQ + 2 * D     # 384 = q + k + v
TC = T // 128        # 16 t-chunks
KC = C // 128        # 8 contraction chunks
NJ = T // 512        # 4 query blocks / groups
EPS = 1.1920929e-7
EXP = mybir.ActivationFunctionType.Exp
SQRT = mybir.ActivationFunctionType.Sqrt


def _bcast_ap(sl, n, at=1):
    ap = list(sl.ap)
    ap.insert(at, [0, n])
    return bass.AP(tensor=sl.tensor, offset=sl.offset, ap=ap)


def _split_waits(nc, maxw=1):
    """Walrus in this toolchain allows 1 sem-wait per instruction; split extras
    onto preceding same-engine NoOps."""
    cnt = 0
    for f in nc.m.functions:
        for b in f.blocks:
            il = list(b.instructions)
            out = []
            changed = False
            for inst in il:
                si = inst.sync_info
                waits = list(si.on_wait) if si and si.on_wait else []
                if len(waits) > maxw:
                    chunks = [waits[i:i + maxw] for i in range(0, len(waits), maxw)]
                    for ch in chunks[:-1]:
                        cnt += 1
                        nop = mybir.InstNoOp(name=f"I-waitfix-{cnt}")
                        nop.engine = inst.engine
                        nop.sync_info = mybir.SyncInfo(on_wait=ch, on_update=[])
                        out.append(nop)
                    si.on_wait = chunks[-1]
                    inst.sync_info = si
                    changed = True
                out.append(inst)
            if changed:
                b.instructions = out
    return cnt


def _build_attn(ctx, tc, outs, ins):
    nc = tc.nc
    xT, wqkv, wo, cos2, sin2 = (
        ins["xT"], ins["wqkv"], ins["wo"], ins["cos2"], ins["sin2"])
    outT = outs["outT"]

    singles = ctx.enter_context(tc.tile_pool(name="singles", bufs=1))

    ident = singles.tile([128, 128], BF16, tag="ident")
    make_identity(nc, ident)
    ones_row = singles.tile([1, 64], F32, tag="ones_row")
    nc.vector.memset(ones_row, 1.0)
    eps_t = singles.tile([128, 1], F32, tag="eps_t")
    nc.vector.memset(eps_t, EPS)

    wqkv_sb = singles.tile([128, KC, DKV], BF16, tag="wqkv_sb")
    wr = wqkv.rearrange("(a p) n -> p a n", p=128)
    xsb = singles.tile([128, KC, T], BF16, tag="xsb")
    xr = xT.rearrange("(a p) t -> p a t", p=128)
    nc.sync.dma_start(out=wqkv_sb[:, 0:2, :], in_=wr[:, 0:2, :])
    nc.sync.dma_start(out=xsb[:, 0:2, 0:128], in_=xr[:, 0:2, 0:128])
    for kc2 in range(2, KC, 2):
        nc.sync.dma_start(out=wqkv_sb[:, kc2:kc2 + 2, :],
                          in_=wr[:, kc2:kc2 + 2, :])
        nc.sync.dma_start(out=xsb[:, kc2:kc2 + 2, 0:128],
                          in_=xr[:, kc2:kc2 + 2, 0:128])
    nc.sync.dma_start(out=xsb[:, 0:2, 128:512], in_=xr[:, 0:2, 128:512])
    nc.sync.dma_start(out=xsb[:, 2:KC, 128:512], in_=xr[:, 2:KC, 128:512])
    cos_sb = singles.tile([128, TC, 32], F32, tag="cos_sb")
    nc.sync.dma_start(out=cos_sb, in_=cos2.rearrange("(a p) d -> p a d", p=128))
    sin_sb = singles.tile([128, TC, 32], F32, tag="sin_sb")
    nc.sync.dma_start(out=sin_sb, in_=sin2.rearrange("(a p) d -> p a d", p=128))
    nc.sync.dma_start(out=xsb[:, :, 512:1024], in_=xr[:, :, 512:1024])
    wo_sb = singles.tile([128, 2, C], BF16, tag="wo_sb")
    nc.sync.dma_start(out=wo_sb, in_=wo.rearrange("(a p) o -> p a o", p=128))
    for r in range(2, NJ):
        nc.sync.dma_start(out=xsb[:, :, r * 512:(r + 1) * 512],
                          in_=xr[:, :, r * 512:(r + 1) * 512])

    q2 = singles.tile([128, TC, DQ], BF16, tag="q2")
    kn = singles.tile([128, TC, 128], BF16, tag="kn")
    v_sb = singles.tile([128, TC, 65], BF16, tag="v_sb")
    nc.vector.memset(v_sb[:, :, 64:65], 1.0)
    qt0 = singles.tile([128, T], BF16, tag="qt0")
    qt1 = singles.tile([128, T], BF16, tag="qt1")
    kt2 = singles.tile([128, T], BF16, tag="kt2")
    yt0 = singles.tile([128, T], BF16, tag="yt0")
    yt1 = singles.tile([128, T], BF16, tag="yt1")
    qts = (qt0, qt1)
    yts = (yt0, yt1)

    mm = ctx.enter_context(tc.tile_pool(name="mm", bufs=2, space="PSUM"))
    s4p = ctx.enter_context(tc.tile_pool(name="s4p", bufs=4, space="PSUM"))
    o65p = ctx.enter_context(tc.tile_pool(name="o65p", bufs=2, space="PSUM"))
    stg = ctx.enter_context(tc.tile_pool(name="stg", bufs=2))
    rt = ctx.enter_context(tc.tile_pool(name="rt", bufs=2))
    ptp = ctx.enter_context(tc.tile_pool(name="ptp", bufs=3))
    smallp = ctx.enter_context(tc.tile_pool(name="smallp", bufs=4))
    osp = ctx.enter_context(tc.tile_pool(name="osp", bufs=8))

    stgts = [None] * NJ
    rts = [None] * NJ

    def qkv_chunk_tasks(g):
        def chunk(c, g=g):
            if c == 0:
                stgts[g] = stg.tile([128, 4, DKV], F32, tag="stg",
                                    name="stgt")
            stgt = stgts[g]
            t = g * 4 + c
            ps = mm.tile([128, 512], F32, tag="mm", name="ps")
            for kc in range(KC):
                nc.tensor.matmul(
                    ps[:, 0:DKV], xsb[:, kc, t * 128:(t + 1) * 128],
                    wqkv_sb[:, kc, :], start=(kc == 0), stop=(kc == KC - 1))
            nc.scalar.copy(stgt[:, c, 0:DQ + 64], ps[:, 0:DQ + 64])
            nc.vector.tensor_copy(v_sb[:, t, 0:64], ps[:, DQ + 64:DKV])
        return [lambda c=c: chunk(c) for c in range(4)]

    def qkv_group(g):
        for f in qkv_chunk_tasks(g):
            f()

    def rope_group_tasks(g):
        return [lambda: rope_rms(g), lambda: rope_q(g), lambda: rope_k(g)]

    def rope_group(g):
        for f in rope_group_tasks(g):
            f()

    def rope_rms(g):
        stgt = stgts[g]
        ts = slice(g * 4, g * 4 + 4)
        q3 = stgt[:, :, 0:DQ].rearrange("p c (h d) -> p c h d", h=HQ)
        k3 = stgt[:, :, DQ:DQ + 64]
        # rms scales from pre-rope values (rotation preserves the norm)
        sq = rt.tile([128, 4, DQ], F32, tag="sq")
        nc.vector.tensor_mul(sq, stgt[:, :, 0:DQ], stgt[:, :, 0:DQ])
        mv = rt.tile([128, 4, HQ], F32, tag="mv")
        nc.vector.tensor_reduce(
            mv, sq.rearrange("p c (h d) -> p c h d", d=D),
            axis=mybir.AxisListType.X, op=mybir.AluOpType.add)
        sd = rt.tile([128, 4, HQ], F32, tag="sd")
        nc.scalar.activation(sd, mv, SQRT, bias=eps_t, scale=1.0 / D)
        rsq = rt.tile([128, 4, HQ], F32, tag="rsq")
        nc.vector.reciprocal(rsq, sd)
        sk = rt.tile([128, 4, 64], F32, tag="sk")
        nc.gpsimd.tensor_mul(sk, k3, k3)
        mk = rt.tile([128, 4, 1], F32, tag="mk")
        nc.vector.tensor_reduce(mk, sk, axis=mybir.AxisListType.X,
                                op=mybir.AluOpType.add)
        sdk = rt.tile([128, 4, 1], F32, tag="sdk")
        nc.scalar.activation(sdk, mk, SQRT, bias=eps_t, scale=1.0 / D)
        rsk = rt.tile([128, 4, 1], F32, tag="rsk")
        nc.vector.reciprocal(rsk, sdk)
        # rms-scaled rotary tables
        cosr = rt.tile([128, 4, HQ, 32], BF16, tag="cosr")
        nc.vector.tensor_mul(cosr, _bcast_ap(cos_sb[:, ts, :], HQ, at=2),
                             _bcast_ap(rsq, 32, at=3))
        sinr = rt.tile([128, 4, HQ, 32], BF16, tag="sinr")
        nc.vector.tensor_mul(sinr, _bcast_ap(sin_sb[:, ts, :], HQ, at=2),
                             _bcast_ap(rsq, 32, at=3))
        rkb = bass.AP(tensor=rsk.tensor, offset=rsk.offset,
                      ap=[rsk.ap[0], rsk.ap[1], [0, 32]])
        cosk = rt.tile([128, 4, 32], BF16, tag="cosk")
        nc.gpsimd.tensor_mul(cosk, cos_sb[:, ts, :], rkb)
        sink = rt.tile([128, 4, 32], BF16, tag="sink")
        nc.gpsimd.tensor_mul(sink, sin_sb[:, ts, :], rkb)
        rts[g] = (cosr, sinr, cosk, sink)

    def rope_q(g):
        stgt = stgts[g]
        ts = slice(g * 4, g * 4 + 4)
        q3 = stgt[:, :, 0:DQ].rearrange("p c (h d) -> p c h d", h=HQ)
        cosr, sinr, cosk, sink = rts[g]
        # rope q -> q2
        x1, x2 = q3[:, :, :, 0:32], q3[:, :, :, 32:64]
        q2v = q2[:, ts, :].rearrange("p c (h d) -> p c h d", h=HQ)
        t1 = rt.tile([128, 4, HQ, 32], BF16, tag="t1")
        t2 = rt.tile([128, 4, HQ, 32], BF16, tag="t2")
        nc.vector.tensor_mul(t1, x1, cosr)
        nc.vector.tensor_mul(t2, x2, sinr)
        nc.vector.tensor_add(q2v[:, :, :, 0:32], t1, t2)
        nc.vector.tensor_mul(t1, x1, sinr)
        nc.vector.tensor_mul(t2, x2, cosr)
        nc.vector.tensor_sub(q2v[:, :, :, 32:64], t2, t1)
    def rope_k(g):
        stgt = stgts[g]
        ts = slice(g * 4, g * 4 + 4)
        k3 = stgt[:, :, DQ:DQ + 64]
        cosr, sinr, cosk, sink = rts[g]
        # rope k -> kn cols 0:64, duplicate to 64:128
        kx1, kx2 = k3[:, :, 0:32], k3[:, :, 32:64]
        u1 = rt.tile([128, 4, 32], BF16, tag="u1")
        u2 = rt.tile([128, 4, 32], BF16, tag="u2")
        nc.gpsimd.tensor_mul(u1, kx1, cosk)
        nc.gpsimd.tensor_mul(u2, kx2, sink)
        nc.gpsimd.tensor_add(kn[:, ts, 0:32], u1, u2)
        nc.gpsimd.tensor_mul(u1, kx1, sink)
        nc.gpsimd.tensor_mul(u2, kx2, cosk)
        nc.gpsimd.tensor_sub(kn[:, ts, 32:64], u2, u1)
        nc.gpsimd.tensor_copy(kn[:, ts, 64:128], kn[:, ts, 0:64])

    def transp_group_tasks(g):
        def tchunk(c, g=g):
            transp_chunk(g, c)
        return [lambda c=c: tchunk(c) for c in range(4)]

    def transp_group(g):
        for f in transp_group_tasks(g):
            f()

    def transp_chunk(g, c):
        for c in [c]:
            t = g * 4 + c
            tp = mm.tile([128, 512], F32, tag="mm")
            tpb = tp.bitcast(BF16)
            nc.tensor.transpose(tpb[:, 0:128], q2[:, t, 0:128], ident)
            nc.tensor.transpose(tpb[:, 128:256], q2[:, t, 128:256], ident)
            nc.tensor.transpose(tpb[:, 256:384], kn[:, t, :], ident)
            sl = slice(t * 128, (t + 1) * 128)
            nc.vector.tensor_copy(qt0[:, sl], tpb[:, 0:128])
            nc.vector.tensor_copy(qt1[:, sl], tpb[:, 128:256])
            nc.vector.tensor_copy(kt2[:, sl], tpb[:, 256:384])

    def attention_tasks(j):
        tasks = []
        pending_ep = []
        jq = j * 512
        npair = 2 * (j + 1)
        for h in range(HQ):
            pair, base = h // 2, (h % 2) * 64
            tpos = (base, 0) if base else None
            st = {"pts": [None] * npair}

            def scores_pair(p, st=st, pair=pair, base=base, tpos=tpos):
                if p == 0:
                    st["o65"] = o65p.tile([65, 512], F32, tag="o65",
                                          name="o65")
                pt = ptp.tile([128, 2, 512], BF16, tag="pt", name="pt")
                st["pts"][p] = pt
                for i2 in range(2):
                    c = 2 * p + i2
                    i_loc = c - 4 * j
                    lo = i_loc * 128 if i_loc > 0 else 0
                    s4 = s4p.tile([128, 512], F32, tag="s4", name="s4")
                    nc.tensor.matmul(
                        s4[:, lo:512],
                        kt2[base:base + 64, c * 128:(c + 1) * 128],
                        qts[pair][base:base + 64, jq + lo:jq + 512],
                        start=True, stop=True, tile_position=tpos)
                    nc.scalar.activation(pt[:, i2, lo:512], s4[:, lo:512],
                                         EXP, scale=0.125)
                    if lo and c == 4 * j + 3:
                        nc.gpsimd.memset(pt[:, i2, 0:lo], 0.0)
                    if i_loc >= 0:
                        nc.gpsimd.affine_select(
                            out=pt[:, i2, lo:lo + 128], in_=pt[:, i2, lo:lo + 128],
                            compare_op=mybir.AluOpType.is_ge, fill=0.0,
                            base=0, pattern=[[1, 128]], channel_multiplier=-1)

            def pv_pair(p, st=st):
                pt = st["pts"][p]
                for i2 in range(2):
                    c = 2 * p + i2
                    i_loc = c - 4 * j
                    last = (c == 4 * j + 3)
                    lo = i_loc * 128 if (i_loc > 0 and not last) else 0
                    nc.tensor.matmul(
                        st["o65"][:, lo:512], v_sb[:, c, :],
                        pt[:, i2, lo:512],
                        start=(c == 0), stop=last)

            def epilogue(st=st, pair=pair, base=base, h=h):
                o65 = st["o65"]
                rec = smallp.tile([1, 512], F32, tag="rec", name="rec")
                nc.vector.reciprocal(rec, o65[64:65, :])
                bc = mm.tile([128, 512], F32, tag="mm", name="bc")
                nc.tensor.matmul(bc[0:64, :], ones_row, rec,
                                 start=True, stop=True)
                bcs = smallp.tile([64, 512], F32, tag="bcs", name="bcs")
                nc.vector.tensor_copy(bcs, bc[0:64, :])
                nc.vector.tensor_mul(
                    yts[pair][base:base + 64, jq:jq + 512], o65[0:64, :], bcs)

            h_tasks = [lambda p=0, f=scores_pair: f(p)]
            for p in range(1, npair):
                h_tasks.append(lambda p=p, f=scores_pair, g=pv_pair:
                               (f(p), g(p - 1)))
            h_tasks.append(lambda f=pv_pair, p=npair - 1: f(p))
            # defer the previous head's epilogue (its bc matmul waits on a
            # DVE reciprocal) until two tasks into this head, so the in-order
            # PE stream does not stall on it
            ins_at = min(2, len(h_tasks))
            tasks.extend(h_tasks[:ins_at])
            if pending_ep:
                tasks.append(pending_ep.pop())
            tasks.extend(h_tasks[ins_at:])
            pending_ep.append(epilogue)
        tasks.append(pending_ep.pop())
        return tasks

    def outproj_tasks(j):
        def mtask(m, j=j):
            op = mm.tile([128, 512], F32, tag="mm", name="op")
            for fc in range(2):
                nc.tensor.matmul(
                    op, wo_sb[:, fc, m * 128:(m + 1) * 128],
                    yts[fc][:, j * 512:(j + 1) * 512],
                    start=(fc == 0), stop=(fc == 1))
            ot = osp.tile([128, 512], BF16, tag="ot", name="ot")
            if j == NJ - 1:
                cp = nc.scalar.copy if (m % 2) else nc.vector.tensor_copy
            else:
                cp = nc.vector.tensor_copy
            cp(ot, op)
            nc.sync.dma_start(
                out=outT[m * 128:(m + 1) * 128, j * 512:(j + 1) * 512],
                in_=ot)
        return [lambda m=m: mtask(m) for m in range(8)]

    def interleave(primary, extra):
        n, m = len(primary), len(extra)
        out, ei = [], 0
        for i, t in enumerate(primary):
            out.append(t)
            while ei < m and ei * n < m * (i + 1):
                out.append(extra[ei])
                ei += 1
        out.extend(extra[ei:])
        return out

    qkv_group(0)
    for task in interleave(qkv_chunk_tasks(1), rope_group_tasks(0)):
        task()
    transp_group(0)
    for g in range(NJ):
        extra = []
        if g >= 1:
            extra += outproj_tasks(g - 1)
        if g < NJ - 2:
            extra += qkv_chunk_tasks(g + 2)
        if g < NJ - 1:
            extra += rope_group_tasks(g + 1)
            extra += transp_group_tasks(g + 1)
        for task in interleave(attention_tasks(g), extra):
            task()
    for task in outproj_tasks(NJ - 1):
        task()


def _build_nc():
    nc = bass.Bass("TRN2", target_bir_lowering=False, debug=False, num_devices=8)
    ins = {
        "xT": nc.dram_tensor("xT", [1024, 2048], BF16, kind="ExternalInput").ap(),
        "wqkv": nc.dram_tensor("wqkv", [1024, DKV], BF16, kind="ExternalInput").ap(),
        "wo": nc.dram_tensor("wo", [256, 1024], BF16, kind="ExternalInput").ap(),
        "cos2": nc.dram_tensor("cos2", [2048, 32], F32, kind="ExternalInput").ap(),
        "sin2": nc.dram_tensor("sin2", [2048, 32], F32, kind="ExternalInput").ap(),
    }
    outs = {"outT": nc.dram_tensor("outT", [1024, 2048], BF16,
                                   kind="ExternalOutput").ap()}
    with TileContext(nc) as tc:
        with ExitStack() as ctx:
            _build_attn(ctx, tc, outs, ins)
    _split_waits(nc, maxw=1)
    return nc


def _shard_inputs(inputs, b, g):
    x, cos, sin = inputs["x"], inputs["cos"], inputs["sin"]
    Wq, Wk, Wv, Wo = inputs["Wq"], inputs["Wk"], inputs["Wv"], inputs["Wo"]
    qs, ks = slice(g * 256, (g + 1) * 256), slice(g * 64, (g + 1) * 64)
    return {
        "xT": np.ascontiguousarray(np.asarray(x[b]).T.astype(NPBF16)),
        "wqkv": np.ascontiguousarray(np.concatenate(
            [np.asarray(Wq[qs]).T, np.asarray(Wk[ks]).T, np.asarray(Wv[ks]).T],
            axis=1).astype(NPBF16)),
        "wo": np.ascontiguousarray(np.asarray(Wo[:, qs]).T.astype(NPBF16)),
        "cos2": np.ascontiguousarray(np.asarray(cos[0, :, 0, :]), dtype=np.float32),
        "sin2": np.ascontiguousarray(np.asarray(sin[0, :, 0, :]), dtype=np.float32),
    }


_STATE = None


def _get_state():
    global _STATE
    if _STATE is not None:
        return _STATE
    import jax
    from jax.sharding import Mesh, PartitionSpec, NamedSharding
    from jax.experimental.shard_map import shard_map
    from concourse.bass2jax import (
        _bass_exec_p, install_neuronx_cc_hook, partition_id_tensor)

    install_neuronx_cc_hook()
    nc = _build_nc()
    pname = nc.partition_id_tensor.name if nc.partition_id_tensor else None

    in_names, out_names, out_avals, zero_outs = [], [], [], []
    for alloc in nc.m.functions[0].allocations:
        if not isinstance(alloc, mybir.MemoryLocationSet):
            continue
        name = alloc.memorylocations[0].name
        if alloc.kind == "ExternalInput":
            if name != pname:
                in_names.append(name)
        elif alloc.kind == "ExternalOutput":
            out_names.append(name)
            shape = tuple(alloc.tensor_shape)
            dtype = mybir.dt.np(alloc.dtype)
            out_avals.append(jax.core.ShapedArray(shape, dtype))
            zero_outs.append(np.zeros(shape, dtype))
    n_params = len(in_names)
    all_names = in_names + out_names
    if pname is not None:
        all_names = all_names + [pname]

    def _body(*args):
        operands = list(args)
        if pname is not None:
            operands.append(partition_id_tensor())
        outs = _bass_exec_p.bind(
            *operands, out_avals=tuple(out_avals), in_names=tuple(all_names),
            out_names=tuple(out_names), lowering_input_output_aliases=(),
            sim_require_finite=True, sim_require_nnan=True, nc=nc)
        return tuple(outs)

    devices = jax.devices()[:8]
    mesh = Mesh(np.asarray(devices), ("core",))
    specs = (PartitionSpec("core"),) * (n_params + 1)
    fn = shard_map(_body, mesh=mesh, in_specs=specs,
                   out_specs=(PartitionSpec("core"),), check_rep=False)
    sharding = NamedSharding(mesh, PartitionSpec("core"))
    zeros = jax.device_put(
        np.zeros((8 * 1024, 2048), NPBF16), sharding)
    dummies = []
    for n in in_names:
        for alloc in nc.m.functions[0].allocations:
            if (isinstance(alloc, mybir.MemoryLocationSet)
                    and alloc.memorylocations[0].name == n):
                shp = tuple(alloc.tensor_shape)
                dt = mybir.dt.np(alloc.dtype)
                dummies.append(jax.device_put(
                    np.zeros((8 * shp[0],) + shp[1:], dt), sharding))
                break
    try:
        from concourse.bass2jax import fast_dispatch_compile
        sharded = fast_dispatch_compile(
            lambda: jax.jit(fn).lower(*dummies, zeros).compile())
    except Exception:
        sharded = jax.jit(fn)
    _STATE = dict(sharded=sharded, sharding=sharding, in_names=in_names,
                  zeros=zeros, jax=jax)
    return _STATE


def _run_device(in_maps):
    st = _get_state()
    jax = st["jax"]
    concat_in = [np.concatenate([m[n] for m in in_maps], axis=0)
                 for n in st["in_names"]]
    dev_in = [jax.device_put(a, st["sharding"]) for a in concat_in]
    out = st["sharded"](*dev_in, st["zeros"])[0]
    return np.asarray(out).reshape(8, 1024, 2048)


def kernel(**inputs) -> np.ndarray:
    inputs = {k: np.asarray(v) for k, v in inputs.items()}
    in_maps = [_shard_inputs(inputs, b, g) for b in range(2) for g in range(4)]
    arr = _run_device(in_maps)
    out = np.zeros((2, 2048, 1024), np.float32)
    for c in range(8):
        out[c // 4] += arr[c].T.astype(np.float32)
    return out


# revision 26
# speedup vs baseline: 1.0024x; 1.0024x over previous
"""Trainium2 Bass kernel: causal GQA self-attention (B=2, T=2048, C=1024,
16 q-heads / 4 kv-heads, rotary + q/k RMS-norm), sharded over 8 NeuronCores
as (batch x kv-group). Self-contained: kernel(**inputs) -> np.ndarray.

Single interleaved pipeline: fused QKV projection, grouped rope/RMS (norms
computed pre-rotation; rotation preserves them), software-pipelined causal
attention with column-trimmed scores/exp, and cross-group task interleaving
to keep the tensor engine ramped. Launch path uses fast-dispatch compile.
"""
import sys
from contextlib import ExitStack

for p in ("/opt/trn_rl_repo", "/root/.axon_site/_ro/trn_rl_repo"):
    if p not in sys.path:
        sys.path.insert(0, p)

import numpy as np
import ml_dtypes

import concourse.bass as bass
import concourse.mybir as mybir
from concourse.tile import TileContext
from concourse.masks import make_identity

F32 = mybir.dt.float32
BF16 = mybir.dt.bfloat16
NPBF16 = ml_dtypes.bfloat16

T, C, HQ, D = 2048, 1024, 4, 64
DQ = HQ * D          # 256 q dims per core
DKV = DQ + 2 * D     # 384 = q + k + v
TC = T // 128        # 16 t-chunks
KC = C // 128        # 8 contraction chunks
NJ = T // 512        # 4 query blocks / groups
EPS = 1.1920929e-7
EXP = mybir.ActivationFunctionType.Exp
SQRT = mybir.ActivationFunctionType.Sqrt


def _bcast_ap(sl, n, at=1):
    ap = list(sl.ap)
    ap.insert(at, [0, n])
    return bass.AP(tensor=sl.tensor, offset=sl.offset, ap=ap)


def _split_waits(nc, maxw=1):
    """Walrus in this toolchain allows 1 sem-wait per instruction; split extras
    onto preceding same-engine NoOps."""
    cnt = 0
    for f in nc.m.functions:
        for b in f.blocks:
            il = list(b.instructions)
            out = []
            changed = False
            for inst in il:
                si = inst.sync_info
                waits = list(si.on_wait) if si and si.on_wait else []
                if len(waits) > maxw:
                    chunks = [waits[i:i + maxw] for i in range(0, len(waits), maxw)]
                    for ch in chunks[:-1]:
                        cnt += 1
                        nop = mybir.InstNoOp(name=f"I-waitfix-{cnt}")
                        nop.engine = inst.engine
                        nop.sync_info = mybir.SyncInfo(on_wait=ch, on_update=[])
                        out.append(nop)
                    si.on_wait = chunks[-1]
                    inst.sync_info = si
                    changed = True
                out.append(inst)
            if changed:
                b.instructions = out
    return cnt


def _build_attn(ctx, tc, outs, ins):
    nc = tc.nc
    xT, wqkv, wo, cos2, sin2 = (
        ins["xT"], ins["wqkv"], ins["wo"], ins["cos2"], ins["sin2"])
    outT = outs["outT"]

    singles = ctx.enter_context(tc.tile_pool(name="singles", bufs=1))

    ident = singles.tile([128, 128], BF16, tag="ident")
    make_identity(nc, ident)
    ones_row = singles.tile([1, 64], F32, tag="ones_row")
    nc.vector.memset(ones_row, 1.0)
    eps_t = singles.tile([128, 1], F32, tag="eps_t")
    nc.vector.memset(eps_t, EPS)

    wqkv_sb = singles.tile([128, KC, DKV], BF16, tag="wqkv_sb")
    wr = wqkv.rearrange("(a p) n -> p a n", p=128)
    xsb = singles.tile([128, KC, T], BF16, tag="xsb")
    xr = xT.rearrange("(a p) t -> p a t", p=128)
    nc.sync.dma_start(out=wqkv_sb[:, 0:2, :], in_=wr[:, 0:2, :])
    nc.sync.dma_start(out=xsb[:, 0:2, 0:128], in_=xr[:, 0:2, 0:128])
    for kc2 in range(2, KC, 2):
        nc.sync.dma_start(out=wqkv_sb[:, kc2:kc2 + 2, :],
                          in_=wr[:, kc2:kc2 + 2, :])
        nc.sync.dma_start(out=xsb[:, kc2:kc2 + 2, 0:128],
                          in_=xr[:, kc2:kc2 + 2, 0:128])
    nc.sync.dma_start(out=xsb[:, 0:2, 128:512], in_=xr[:, 0:2, 128:512])
    nc.sync.dma_start(out=xsb[:, 2:KC, 128:512], in_=xr[:, 2:KC, 128:512])
    cos_sb = singles.tile([128, TC, 32], F32, tag="cos_sb")
    nc.sync.dma_start(out=cos_sb, in_=cos2.rearrange("(a p) d -> p a d", p=128))
    sin_sb = singles.tile([128, TC, 32], F32, tag="sin_sb")
    nc.sync.dma_start(out=sin_sb, in_=sin2.rearrange("(a p) d -> p a d", p=128))
    nc.sync.dma_start(out=xsb[:, :, 512:1024], in_=xr[:, :, 512:1024])
    wo_sb = singles.tile([128, 2, C], BF16, tag="wo_sb")
    nc.sync.dma_start(out=wo_sb, in_=wo.rearrange("(a p) o -> p a o", p=128))
    for r in range(2, NJ):
        nc.sync.dma_start(out=xsb[:, :, r * 512:(r + 1) * 512],
                          in_=xr[:, :, r * 512:(r + 1) * 512])

    q2 = singles.tile([128, TC, DQ], BF16, tag="q2")
    kn = singles.tile([128, TC, 128], BF16, tag="kn")
    v_sb = singles.tile([128, TC, 65], BF16, tag="v_sb")
    nc.vector.memset(v_sb[:, :, 64:65], 1.0)
    qt0 = singles.tile([128, T], BF16, tag="qt0")
    qt1 = singles.tile([128, T], BF16, tag="qt1")
    kt2 = singles.tile([128, T], BF16, tag="kt2")
    yt0 = singles.tile([128, T], BF16, tag="yt0")
    yt1 = singles.tile([128, T], BF16, tag="yt1")
    qts = (qt0, qt1)
    yts = (yt0, yt1)

    mm = ctx.enter_context(tc.tile_pool(name="mm", bufs=2, space="PSUM"))
    s4p = ctx.enter_context(tc.tile_pool(name="s4p", bufs=4, space="PSUM"))
    o65p = ctx.enter_context(tc.tile_pool(name="o65p", bufs=2, space="PSUM"))
    stg = ctx.enter_context(tc.tile_pool(name="stg", bufs=2))
    rt = ctx.enter_context(tc.tile_pool(name="rt", bufs=2))
    ptp = ctx.enter_context(tc.tile_pool(name="ptp", bufs=3))
    smallp = ctx.enter_context(tc.tile_pool(name="smallp", bufs=4))
    osp = ctx.enter_context(tc.tile_pool(name="osp", bufs=8))

    stgts = [None] * NJ
    rts = [None] * NJ

    def qkv_chunk_tasks(g):
        def chunk(c, g=g):
            if c == 0:
                stgts[g] = stg.tile([128, 4, DKV], F32, tag="stg",
                                    name="stgt")
            stgt = stgts[g]
            t = g * 4 + c
            ps = mm.tile([128, 512], F32, tag="mm", name="ps")
            for kc in range(KC):
                nc.tensor.matmul(
                    ps[:, 0:DKV], xsb[:, kc, t * 128:(t + 1) * 128],
                    wqkv_sb[:, kc, :], start=(kc == 0), stop=(kc == KC - 1))
            nc.scalar.copy(stgt[:, c, 0:DQ + 64], ps[:, 0:DQ + 64])
            nc.vector.tensor_copy(v_sb[:, t, 0:64], ps[:, DQ + 64:DKV])
        return [lambda c=c: chunk(c) for c in range(4)]

    def qkv_group(g):
        for f in qkv_chunk_tasks(g):
            f()

    def rope_group_tasks(g):
        return [lambda: rope_rms(g), lambda: rope_q(g), lambda: rope_k(g)]

    def rope_group(g):
        for f in rope_group_tasks(g):
            f()

    def rope_rms(g):
        stgt = stgts[g]
        ts = slice(g * 4, g * 4 + 4)
        q3 = stgt[:, :, 0:DQ].rearrange("p c (h d) -> p c h d", h=HQ)
        k3 = stgt[:, :, DQ:DQ + 64]
        # rms scales from pre-rope values (rotation preserves the norm)
        sq = rt.tile([128, 4, DQ], F32, tag="sq")
        nc.vector.tensor_mul(sq, stgt[:, :, 0:DQ], stgt[:, :, 0:DQ])
        mv = rt.tile([128, 4, HQ], F32, tag="mv")
        nc.vector.tensor_reduce(
            mv, sq.rearrange("p c (h d) -> p c h d", d=D),
            axis=mybir.AxisListType.X, op=mybir.AluOpType.add)
        sd = rt.tile([128, 4, HQ], F32, tag="sd")
        nc.scalar.activation(sd, mv, SQRT, bias=eps_t, scale=1.0 / D)
        rsq = rt.tile([128, 4, HQ], F32, tag="rsq")
        nc.vector.reciprocal(rsq, sd)
        sk = rt.tile([128, 4, 64], F32, tag="sk")
        nc.gpsimd.tensor_mul(sk, k3, k3)
        mk = rt.tile([128, 4, 1], F32, tag="mk")
        nc.vector.tensor_reduce(mk, sk, axis=mybir.AxisListType.X,
                                op=mybir.AluOpType.add)
        sdk = rt.tile([128, 4, 1], F32, tag="sdk")
        nc.scalar.activation(sdk, mk, SQRT, bias=eps_t, scale=1.0 / D)
        rsk = rt.tile([128, 4, 1], F32, tag="rsk")
        nc.vector.reciprocal(rsk, sdk)
        # rms-scaled rotary tables
        cosr = rt.tile([128, 4, HQ, 32], BF16, tag="cosr")
        nc.vector.tensor_mul(cosr, _bcast_ap(cos_sb[:, ts, :], HQ, at=2),
                             _bcast_ap(rsq, 32, at=3))
        sinr = rt.tile([128, 4, HQ, 32], BF16, tag="sinr")
        nc.vector.tensor_mul(sinr, _bcast_ap(sin_sb[:, ts, :], HQ, at=2),
                             _bcast_ap(rsq, 32, at=3))
        rkb = bass.AP(tensor=rsk.tensor, offset=rsk.offset,
                      ap=[rsk.ap[0], rsk.ap[1], [0, 32]])
        cosk = rt.tile([128, 4, 32], BF16, tag="cosk")
        nc.gpsimd.tensor_mul(cosk, cos_sb[:, ts, :], rkb)
        sink = rt.tile([128, 4, 32], BF16, tag="sink")
        nc.gpsimd.tensor_mul(sink, sin_sb[:, ts, :], rkb)
        rts[g] = (cosr, sinr, cosk, sink)

    def rope_q(g):
        stgt = stgts[g]
        ts = slice(g * 4, g * 4 + 4)
        q3 = stgt[:, :, 0:DQ].rearrange("p c (h d) -> p c h d", h=HQ)
        cosr, sinr, cosk, sink = rts[g]
        # rope q -> q2
        x1, x2 = q3[:, :, :, 0:32], q3[:, :, :, 32:64]
        q2v = q2[:, ts, :].rearrange("p c (h d) -> p c h d", h=HQ)
        t1 = rt.tile([128, 4, HQ, 32], BF16, tag="t1")
        t2 = rt.tile([128, 4, HQ, 32], BF16, tag="t2")
        nc.vector.tensor_mul(t1, x1, cosr)
        nc.vector.tensor_mul(t2, x2, sinr)
        nc.vector.tensor_add(q2v[:, :, :, 0:32], t1, t2)
        nc.vector.tensor_mul(t1, x1, sinr)
        nc.vector.tensor_mul(t2, x2, cosr)
        nc.vector.tensor_sub(q2v[:, :, :, 32:64], t2, t1)
    def rope_k(g):
        stgt = stgts[g]
        ts = slice(g * 4, g * 4 + 4)
        k3 = stgt[:, :, DQ:DQ + 64]
        cosr, sinr, cosk, sink = rts[g]
        # rope k -> kn cols 0:64, duplicate to 64:128
        kx1, kx2 = k3[:, :, 0:32], k3[:, :, 32:64]
        u1 = rt.tile([128, 4, 32], BF16, tag="u1")
        u2 = rt.tile([128, 4, 32], BF16, tag="u2")
        nc.gpsimd.tensor_mul(u1, kx1, cosk)
        nc.gpsimd.tensor_mul(u2, kx2, sink)
        nc.gpsimd.tensor_add(kn[:, ts, 0:32], u1, u2)
        nc.gpsimd.tensor_mul(u1, kx1, sink)
        nc.gpsimd.tensor_mul(u2, kx2, cosk)
        nc.gpsimd.tensor_sub(kn[:, ts, 32:64], u2, u1)
        nc.gpsimd.tensor_copy(kn[:, ts, 64:128], kn[:, ts, 0:64])

    def transp_group_tasks(g):
        def tchunk(c, g=g):
            transp_chunk(g, c)
        return [lambda c=c: tchunk(c) for c in range(4)]

    def transp_group(g):
        for f in transp_group_tasks(g):
            f()

    def transp_chunk(g, c):
        for c in [c]:
            t = g * 4 + c
            tp = mm.tile([128, 512], F32, tag="mm")
            tpb = tp.bitcast(BF16)
            nc.tensor.transpose(tpb[:, 0:128], q2[:, t, 0:128], ident)
            nc.tensor.transpose(tpb[:, 128:256], q2[:, t, 128:256], ident)
            nc.tensor.transpose(tpb[:, 256:384], kn[:, t, :], ident)
            sl = slice(t * 128, (t + 1) * 128)
            nc.vector.tensor_copy(qt0[:, sl], tpb[:, 0:128])
            nc.vector.tensor_copy(qt1[:, sl], tpb[:, 128:256])
            nc.vector.tensor_copy(kt2[:, sl], tpb[:, 256:384])

    def attention_tasks(j):
        tasks = []
        pending_ep = []
        jq = j * 512
        npair = 2 * (j + 1)
        for h in range(HQ):
            pair, base = h // 2, (h % 2) * 64
            tpos = (base, 0) if base else None
            st = {"pts": [None] * npair}

            def scores_pair(p, st=st, pair=pair, base=base, tpos=tpos):
                if p == 0:
                    st["o65"] = o65p.tile([65, 512], F32, tag="o65",
                                          name="o65")
                pt = ptp.tile([128, 2, 512], BF16, tag="pt", name="pt")
                st["pts"][p] = pt
                for i2 in range(2):
                    c = 2 * p + i2
                    i_loc = c - 4 * j
                    lo = i_loc * 128 if i_loc > 0 else 0
                    s4 = s4p.tile([128, 512], F32, tag="s4", name="s4")
                    nc.tensor.matmul(
                        s4[:, lo:512],
                        kt2[base:base + 64, c * 128:(c + 1) * 128],
                        qts[pair][base:base + 64, jq + lo:jq + 512],
                        start=True, stop=True, tile_position=tpos)
                    nc.scalar.activation(pt[:, i2, lo:512], s4[:, lo:512],
                                         EXP, scale=0.125)
                    if lo and c == 4 * j + 3 and j == 0:
                        nc.gpsimd.memset(pt[:, i2, 0:lo], 0.0)
                    if i_loc >= 0:
                        nc.gpsimd.affine_select(
                            out=pt[:, i2, lo:lo + 128], in_=pt[:, i2, lo:lo + 128],
                            compare_op=mybir.AluOpType.is_ge, fill=0.0,
                            base=0, pattern=[[1, 128]], channel_multiplier=-1)

            def pv_pair(p, st=st):
                pt = st["pts"][p]
                for i2 in range(2):
                    c = 2 * p + i2
                    i_loc = c - 4 * j
                    if j >= 1 and i_loc == 0:
                        continue  # deferred below; carries the group stop
                    last = (c == 4 * j + 3)
                    if j >= 1:
                        lo = i_loc * 128 if i_loc > 0 else 0
                        stop = False
                    else:
                        lo = i_loc * 128 if (i_loc > 0 and not last) else 0
                        stop = last
                    nc.tensor.matmul(
                        st["o65"][:, lo:512], v_sb[:, c, :],
                        pt[:, i2, lo:512],
                        start=(c == 0), stop=stop)

            def pv_tail(st=st):
                # j>=1: the full-width diagonal chunk closes the group
                pt = st["pts"][(4 * j) // 2]
                nc.tensor.matmul(st["o65"], v_sb[:, 4 * j, :], pt[:, 0, :],
                                 start=False, stop=True)

            def epilogue(st=st, pair=pair, base=base, h=h):
                o65 = st["o65"]
                rec = smallp.tile([1, 512], F32, tag="rec", name="rec")
                nc.vector.reciprocal(rec, o65[64:65, :])
                bc = mm.tile([128, 512], F32, tag="mm", name="bc")
                nc.tensor.matmul(bc[0:64, :], ones_row, rec,
                                 start=True, stop=True)
                bcs = smallp.tile([64, 512], F32, tag="bcs", name="bcs")
                nc.vector.tensor_copy(bcs, bc[0:64, :])
                nc.vector.tensor_mul(
                    yts[pair][base:base + 64, jq:jq + 512], o65[0:64, :], bcs)

            h_tasks = [lambda p=0, f=scores_pair: f(p)]
            for p in range(1, npair):
                h_tasks.append(lambda p=p, f=scores_pair, g=pv_pair:
                               (f(p), g(p - 1)))
            h_tasks.append(lambda f=pv_pair, p=npair - 1: f(p))
            if j >= 1:
                h_tasks.append(pv_tail)
            # defer the previous head's epilogue (its bc matmul waits on a
            # DVE reciprocal) until two tasks into this head, so the in-order
            # PE stream does not stall on it
            ins_at = min(2, len(h_tasks))
            tasks.extend(h_tasks[:ins_at])
            if pending_ep:
                tasks.append(pending_ep.pop())
            tasks.extend(h_tasks[ins_at:])
            pending_ep.append(epilogue)
        tasks.append(pending_ep.pop())
        return tasks

    def outproj_tasks(j):
        def mtask(m, j=j):
            op = mm.tile([128, 512], F32, tag="mm", name="op")
            for fc in range(2):
                nc.tensor.matmul(
                    op, wo_sb[:, fc, m * 128:(m + 1) * 128],
                    yts[fc][:, j * 512:(j + 1) * 512],
                    start=(fc == 0), stop=(fc == 1))
            ot = osp.tile([128, 512], BF16, tag="ot", name="ot")
            if j == NJ - 1:
                cp = nc.scalar.copy if (m % 2) else nc.vector.tensor_copy
            else:
                cp = nc.vector.tensor_copy
            cp(ot, op)
            nc.sync.dma_start(
                out=outT[m * 128:(m + 1) * 128, j * 512:(j + 1) * 512],
                in_=ot)
        return [lambda m=m: mtask(m) for m in range(8)]

    def interleave(primary, extra):
        n, m = len(primary), len(extra)
        out, ei = [], 0
        for i, t in enumerate(primary):
            out.append(t)
            while ei < m and ei * n < m * (i + 1):
                out.append(extra[ei])
                ei += 1
        out.extend(extra[ei:])
        return out

    qkv_group(0)
    for task in interleave(qkv_chunk_tasks(1), rope_group_tasks(0)):
        task()
    transp_group(0)
    for g in range(NJ):
        extra = []
        if g >= 1:
            extra += outproj_tasks(g - 1)
        if g < NJ - 2:
            extra += qkv_chunk_tasks(g + 2)
        if g < NJ - 1:
            extra += rope_group_tasks(g + 1)
            extra += transp_group_tasks(g + 1)
        for task in interleave(attention_tasks(g), extra):
            task()
    for task in outproj_tasks(NJ - 1):
        task()


def _build_nc():
    nc = bass.Bass("TRN2", target_bir_lowering=False, debug=False, num_devices=8)
    ins = {
        "xT": nc.dram_tensor("xT", [1024, 2048], BF16, kind="ExternalInput").ap(),
        "wqkv": nc.dram_tensor("wqkv", [1024, DKV], BF16, kind="ExternalInput").ap(),
        "wo": nc.dram_tensor("wo", [256, 1024], BF16, kind="ExternalInput").ap(),
        "cos2": nc.dram_tensor("cos2", [2048, 32], F32, kind="ExternalInput").ap(),
        "sin2": nc.dram_tensor("sin2", [2048, 32], F32, kind="ExternalInput").ap(),
    }
    outs = {"outT": nc.dram_tensor("outT", [1024, 2048], BF16,
                                   kind="ExternalOutput").ap()}
    with TileContext(nc) as tc:
        with ExitStack() as ctx:
            _build_attn(ctx, tc, outs, ins)
    _split_waits(nc, maxw=1)
    return nc


def _shard_inputs(inputs, b, g):
    x, cos, sin = inputs["x"], inputs["cos"], inputs["sin"]
    Wq, Wk, Wv, Wo = inputs["Wq"], inputs["Wk"], inputs["Wv"], inputs["Wo"]
    qs, ks = slice(g * 256, (g + 1) * 256), slice(g * 64, (g + 1) * 64)
    return {
        "xT": np.ascontiguousarray(np.asarray(x[b]).T.astype(NPBF16)),
        "wqkv": np.ascontiguousarray(np.concatenate(
            [np.asarray(Wq[qs]).T, np.asarray(Wk[ks]).T, np.asarray(Wv[ks]).T],
            axis=1).astype(NPBF16)),
        "wo": np.ascontiguousarray(np.asarray(Wo[:, qs]).T.astype(NPBF16)),
        "cos2": np.ascontiguousarray(np.asarray(cos[0, :, 0, :]), dtype=np.float32),
        "sin2": np.ascontiguousarray(np.asarray(sin[0, :, 0, :]), dtype=np.float32),
    }


_STATE = None


def _get_state():
    global _STATE
    if _STATE is not None:
        return _STATE
    import jax
    from jax.sharding import Mesh, PartitionSpec, NamedSharding
    from jax.experimental.shard_map import shard_map
    from concourse.bass2jax import (
        _bass_exec_p, install_neuronx_cc_hook, partition_id_tensor)

    install_neuronx_cc_hook()
    nc = _build_nc()
    pname = nc.partition_id_tensor.name if nc.partition_id_tensor else None

    in_names, out_names, out_avals, zero_outs = [], [], [], []
    for alloc in nc.m.functions[0].allocations:
        if not isinstance(alloc, mybir.MemoryLocationSet):
            continue
        name = alloc.memorylocations[0].name
        if alloc.kind == "ExternalInput":
            if name != pname:
                in_names.append(name)
        elif alloc.kind == "ExternalOutput":
            out_names.append(name)
            shape = tuple(alloc.tensor_shape)
            dtype = mybir.dt.np(alloc.dtype)
            out_avals.append(jax.core.ShapedArray(shape, dtype))
            zero_outs.append(np.zeros(shape, dtype))
    n_params = len(in_names)
    all_names = in_names + out_names
    if pname is not None:
        all_names = all_names + [pname]

    def _body(*args):
        operands = list(args)
        if pname is not None:
            operands.append(partition_id_tensor())
        outs = _bass_exec_p.bind(
            *operands, out_avals=tuple(out_avals), in_names=tuple(all_names),
            out_names=tuple(out_names), lowering_input_output_aliases=(),
            sim_require_finite=True, sim_require_nnan=True, nc=nc)
        return tuple(outs)

    devices = jax.devices()[:8]
    mesh = Mesh(np.asarray(devices), ("core",))
    specs = (PartitionSpec("core"),) * (n_params + 1)
    fn = shard_map(_body, mesh=mesh, in_specs=specs,
                   out_specs=(PartitionSpec("core"),), check_rep=False)
    sharding = NamedSharding(mesh, PartitionSpec("core"))
    zeros = jax.device_put(
        np.zeros((8 * 1024, 2048), NPBF16), sharding)
    dummies = []
    for n in in_names:
        for alloc in nc.m.functions[0].allocations:
            if (isinstance(alloc, mybir.MemoryLocationSet)
                    and alloc.memorylocations[0].name == n):
                shp = tuple(alloc.tensor_shape)
                dt = mybir.dt.np(alloc.dtype)
                dummies.append(jax.device_put(
                    np.zeros((8 * shp[0],) + shp[1:], dt), sharding))
                break
    try:
        from concourse.bass2jax import fast_dispatch_compile
        sharded = fast_dispatch_compile(
            lambda: jax.jit(fn).lower(*dummies, zeros).compile())
    except Exception:
        sharded = jax.jit(fn)
    _STATE = dict(sharded=sharded, sharding=sharding, in_names=in_names,
                  zeros=zeros, jax=jax)
    return _STATE


def _run_device(in_maps):
    st = _get_state()
    jax = st["jax"]
    concat_in = [np.concatenate([m[n] for m in in_maps], axis=0)
                 for n in st["in_names"]]
    dev_in = [jax.device_put(a, st["sharding"]) for a in concat_in]
    out = st["sharded"](*dev_in, st["zeros"])[0]
    return np.asarray(out).reshape(8, 1024, 2048)


def kernel(**inputs) -> np.ndarray:
    inputs = {k: np.asarray(v) for k, v in inputs.items()}
    in_maps = [_shard_inputs(inputs, b, g) for b in range(2) for g in range(4)]
    arr = _run_device(in_maps)
    out = np.zeros((2, 2048, 1024), np.float32)
    for c in range(8):
        out[c // 4] += arr[c].T.astype(np.float32)
    return out
